# revision 1
# baseline (speedup 1.0000x reference)
"""Trainium2 Bass kernel for nn_EntropyLM (wavelet-coeff mixer + chunked MHA + output proj).

Data-parallel over the 16 independent (batch x chunk) blocks, 2 per core.

Precision plan (validated in numpy, predicted rel-err ~8.4e-3 vs 2e-2 gate):
  * Mixer path (coef, w1, w2) and output projection in fp16 on the PE.
  * Attention path (q/k/v, scores, PV, wo) in fp8-e4m3 with DoubleRow
    matmuls: two stacked 128-deep K-subtiles at 0.5 cycles/row.
  * fp8 range handling: weights pre-scaled by 64, activations rescaled on
    PSUM evacuation; the residual branch carries an 8192x scale that LN2
    absorbs (eps scaled to match); the softmax-denominator ones vector is
    1/32 so PV normalization applies the ocat fp8 range scale for free.
  * Residual add (wo_out + mixed) is done on the PE by accumulating an
    identity matmul of mixN into the wo PSUM group; LN2 stats and z read
    straight from PSUM, so `res` never materializes in SBUF.

Scheduling: the two chunks are software-pipelined by emission order (engine
queues are FIFO): chunk1's mixer stages (PE/DVE-heavy) are emitted inside
chunk0's attention window (Act/exp-bound), and chunk1's attention overlaps
chunk0's output stages.  PSUM: 512-wide `ps` pool for GEMMs interleaved
with attention; 1024-wide `ps2` pool for q/k/v GEMMs, score tiles, and the
wo+LN2 groups (freed per-token-tile so rotation never clobbers live data).
"""

import numpy as np
import ml_dtypes

B, S, H, G, W = 4, 4096, 1024, 256, 8
CHUNK = 1024
NUM_HEADS = 4
HD = H // NUM_HEADS          # 256 per-head dim
HM = H // 2                  # 512 mixer hidden
N_CHUNKS = B * (S // CHUNK)  # 16 independent chunks
N_CORES = 8
CPC = N_CHUNKS // N_CORES    # 2 chunks per core
NT = CHUNK // 128            # 8 token tiles
KH = H // 128                # 8 feature tiles (H)
KM = HM // 128               # 4 feature tiles (HM)
KP = KH // 2                 # 4 double-row K pairs over H
EPS = 1e-5
SC_RES = 8192.0              # residual-branch scale, absorbed by LN2
SC_MIX8 = 4.0                # fp8 storage scale for mixed
SC_W = 64.0                  # fp8 weight scale
SC_OT = 32.0                 # ocat fp8 range scale (via ones = 1/32)
FP16 = np.float16
FP8 = ml_dtypes.float8_e4m3

_COMPILED = None


def _build():
    import concourse.bass as bass  # noqa: F401
    import concourse.tile as tile
    from concourse import bacc, mybir

    f16 = mybir.dt.float16
    f8 = mybir.dt.float8e4
    f32 = mybir.dt.float32
    Alu = mybir.AluOpType
    Act = mybir.ActivationFunctionType
    DR = mybir.MatmulPerfMode.DoubleRow

    nc = bacc.Bacc("TRN2", target_bir_lowering=False, debug=False,
                   enable_asserts=True, num_devices=N_CORES)

    xt = nc.dram_tensor("xt", [CPC, H, CHUNK], f16, kind="ExternalInput")
    kernT = nc.dram_tensor("kernt", [H, W], f16, kind="ExternalInput")
    w1a = nc.dram_tensor("w1a", [W + 1, HM], f16, kind="ExternalInput")
    gln = nc.dram_tensor("gln", [128, KM], f32, kind="ExternalInput")
    bln = nc.dram_tensor("bln", [128, KM], f32, kind="ExternalInput")
    w2 = nc.dram_tensor("w2", [HM, H], f16, kind="ExternalInput")
    b2c = nc.dram_tensor("b2c", [128, KH], f32, kind="ExternalInput")
    b2r = nc.dram_tensor("b2r", [128, KH], f32, kind="ExternalInput")
    wq8 = nc.dram_tensor("wq8", [H, H], f8, kind="ExternalInput")
    wk8 = nc.dram_tensor("wk8", [H, H], f8, kind="ExternalInput")
    wv8 = nc.dram_tensor("wv8", [H, H], f8, kind="ExternalInput")
    wo8 = nc.dram_tensor("wo8", [H, H], f8, kind="ExternalInput")
    identD = nc.dram_tensor("ident", [128, 128], f16, kind="ExternalInput")
    gw = nc.dram_tensor("gw", [H, G], f16, kind="ExternalInput")
    bw = nc.dram_tensor("bw", [128, G], f16, kind="ExternalInput")
    y = nc.dram_tensor("y", [CPC, CHUNK, G], f32, kind="ExternalOutput")

    with tile.TileContext(nc) as tc:
        with (
            tc.tile_pool(name="wp", bufs=1) as wp,
            tc.tile_pool(name="ws", bufs=1) as ws,
            tc.tile_pool(name="sm", bufs=4) as sm,
            tc.tile_pool(name="ps", bufs=2, space="PSUM") as ps,
            tc.tile_pool(name="ps2", bufs=3, space="PSUM") as ps2,
        ):
            # ---------- persistent weights ----------
            kt_sb = wp.tile([128, KH, W], f16, tag="ktw")
            nc.sync.dma_start(kt_sb[:], kernT.ap().rearrange("(i p) w -> p i w", p=128))
            w1a_sb = wp.tile([W + 1, HM], f16, tag="w1a")
            nc.sync.dma_start(w1a_sb[:], w1a.ap())
            gln_sb = wp.tile([128, KM], f32, tag="gln")
            nc.sync.dma_start(gln_sb[:], gln.ap())
            bln_sb = wp.tile([128, KM], f32, tag="bln")
            nc.sync.dma_start(bln_sb[:], bln.ap())
            b2_sb = wp.tile([128, KH], f32, tag="b2")
            nc.sync.dma_start(b2_sb[:], b2c.ap())
            b2r_sb = wp.tile([128, KH], f32, tag="b2r")
            nc.sync.dma_start(b2r_sb[:], b2r.ap())
            w2_sb = wp.tile([128, KM, H], f16, tag="w2s")
            wq_sb = wp.tile([128, KH, H], f8, tag="wq")
            wk_sb = wp.tile([128, KH, H], f8, tag="wk")
            wv_sb = wp.tile([128, KH, H], f8, tag="wv")
            wo_sb = wp.tile([128, KH, H], f8, tag="wo")
            id_sb = wp.tile([128, 128], f16, tag="ident")
            bw_sb = wp.tile([128, G], f16, tag="bw")

            def load_weights():
                # emitted after the input loads so x doesn't queue behind 6MB
                nc.sync.dma_start(w2_sb[:], w2.ap().rearrange("(i p) m -> p i m", p=128))
                nc.sync.dma_start(wq_sb[:], wq8.ap().rearrange("(i p) m -> p i m", p=128))
                nc.sync.dma_start(wk_sb[:], wk8.ap().rearrange("(i p) m -> p i m", p=128))
                nc.sync.dma_start(wv_sb[:], wv8.ap().rearrange("(i p) m -> p i m", p=128))
                nc.sync.dma_start(wo_sb[:], wo8.ap().rearrange("(i p) m -> p i m", p=128))
                nc.sync.dma_start(id_sb[:], identD.ap())
                nc.sync.dma_start(bw_sb[:], bw.ap())
            ones2 = wp.tile([128, 2, 1], f8, tag="ones2")
            nc.vector.memset(ones2[:], 1.0 / SC_OT)
            eps_sb = wp.tile([128, 1], f32, tag="eps")
            nc.vector.memset(eps_sb[:], EPS)
            eps2_sb = wp.tile([128, 1], f32, tag="eps2")
            nc.vector.memset(eps2_sb[:], EPS * SC_RES * SC_RES)

            St = [dict() for _ in range(CPC)]
            Gw = {}

            def load_gw():
                # gw time-shares the hidT slot (dead once w2(1) is emitted);
                # padded to the tag's 8KB byte size
                gwp = ws.tile([128, KH, 2, G], f16, tag="hidT", bufs=1,
                              name="gwpad")
                nc.sync.dma_start(gwp[:, :, 0, :],
                                  gw.ap().rearrange("(i p) g -> p i g", p=128))
                Gw["t"] = gwp

            # ---------- stage 1: input load + wavelet coeffs ----------
            def st1(c):
                xts = ws.tile([128, KH, CHUNK], f16, tag="A", bufs=2)
                for ii in range(KH):
                    nc.sync.dma_start(
                        xts[:, ii:ii + 1, :],
                        xt.ap()[c, ii * 128:(ii + 1) * 128, :].rearrange(
                            "(i p) t -> p i t", p=128))
                coef = ws.tile([W + 1, CHUNK], f16, tag="coef", bufs=2)
                nc.gpsimd.memset(coef[:, :], 1.0)  # row W = folded mix_b1
                for n in range(2):
                    cps = ps.tile([128, 512], f32, tag="mm")
                    for i in range(KH):
                        nc.tensor.matmul(cps[:W, :], kt_sb[:, i, :],
                                         xts[:, i, n * 512:(n + 1) * 512],
                                         start=(i == 0), stop=(i == KH - 1))
                    nc.scalar.copy(coef[:W, n * 512:(n + 1) * 512], cps[:W, :])
                St[c]["coef"] = coef

            # ---------- stage 2a (c1): w1 + stats, hid evacuated to fp16 ----------
            # Pays the cross-engine LN latency early (in the head window) so
            # st2b's sqrts are dependency-ready and don't stall the Act queue.
            def st2a(c):
                coef = St[c]["coef"]
                hid16 = ws.tile([128, NT, HM], f16, tag="O8y", bufs=1)
                mvs = ws.tile([128, NT, 2], f32, tag="MVS", bufs=1)
                ivs = ws.tile([128, NT, 1], f32, tag="IVS", bufs=2,
                              name="ivs") if c == 0 else None
                for t in range(NT):
                    hps = ps.tile([128, HM], f32, tag="mm")
                    nc.tensor.matmul(hps[:], coef[:, t * 128:(t + 1) * 128],
                                     w1a_sb[:], start=True, stop=True)
                    st6 = sm.tile([128, 6], f32, tag="st6")
                    nc.vector.bn_stats(st6[:], hps[:])
                    nc.vector.bn_aggr(mvs[:, t, :], st6[:])
                    nc.vector.tensor_copy(hid16[:, t, :], hps[:])
                    if c == 0:
                        # Act is idle in the head: finish the whole iv chain
                        # here so st2b is pure DVE+DMA
                        sq = sm.tile([128, 1], f32, tag="sq")
                        nc.scalar.activation(sq[:], mvs[:, t, 1:2], Act.Sqrt,
                                             bias=eps_sb[:])
                        nc.vector.reciprocal(ivs[:, t, :], sq[:])
                St[c]["hid16"], St[c]["mvs"] = hid16, mvs
                St[c]["ivs"] = ivs

            def st2b(c):
                hid16, mvs = St[c]["hid16"], St[c]["mvs"]
                hidT = ws.tile([128, KM, CHUNK], f16, tag="hidT", bufs=1)
                for t in range(NT):
                    if c == 0:
                        iv = St[c]["ivs"][:, t, :]
                    else:
                        sq = sm.tile([128, 1], f32, tag="sq")
                        nc.scalar.activation(sq[:], mvs[:, t, 1:2], Act.Sqrt,
                                             bias=eps_sb[:])
                        iv = sm.tile([128, 1], f32, tag="iv")
                        nc.vector.reciprocal(iv[:], sq[:])
                    tmp = sm.tile([128, HM], f16, tag="mtmp")
                    nc.vector.tensor_scalar(tmp[:], hid16[:, t, :],
                                            mvs[:, t, 0:1], iv[:],
                                            op0=Alu.subtract, op1=Alu.mult)
                    nc.sync.dma_start_transpose(hidT[:, :, t * 128:(t + 1) * 128],
                                                tmp[:])
                for nh in range(2):
                    for ki in range(KM):
                        sl = hidT[:, ki, nh * 512:(nh + 1) * 512]
                        nc.scalar.activation(sl, sl, Act.Gelu,
                                             bias=bln_sb[:, ki:ki + 1],
                                             scale=gln_sb[:, ki:ki + 1])
                St[c]["hidT"] = hidT

            # ---------- stage 2: w1 + LN1 + gelu -> hidT ----------
            def st2(c):
                coef = St[c]["coef"]
                hidT = ws.tile([128, KM, CHUNK], f16, tag="hidT", bufs=1)
                for t in range(NT):
                    hps = ps.tile([128, HM], f32, tag="mm")
                    nc.tensor.matmul(hps[:], coef[:, t * 128:(t + 1) * 128],
                                     w1a_sb[:], start=True, stop=True)
                    st6 = sm.tile([128, 6], f32, tag="st6")
                    nc.vector.bn_stats(st6[:], hps[:])
                    mv = sm.tile([128, 2], f32, tag="mv")
                    nc.vector.bn_aggr(mv[:], st6[:])
                    sq = sm.tile([128, 1], f32, tag="sq")
                    nc.scalar.activation(sq[:], mv[:, 1:2], Act.Sqrt, bias=eps_sb[:])
                    iv = sm.tile([128, 1], f32, tag="iv")
                    nc.vector.reciprocal(iv[:], sq[:])
                    tmp = sm.tile([128, HM], f16, tag="mtmp")
                    if c == 0 and t % 2 == 1:
                        nmi1 = sm.tile([128, 1], f32, tag="nmi")
                        nc.vector.tensor_scalar(nmi1[:], mv[:, 0:1], iv[:], -1.0,
                                                op0=Alu.mult, op1=Alu.mult)
                        nc.scalar.activation(tmp[:], hps[:], Act.Identity,
                                             bias=nmi1[:], scale=iv[:])
                    else:
                        nc.vector.tensor_scalar(tmp[:], hps[:], mv[:, 0:1], iv[:],
                                                op0=Alu.subtract, op1=Alu.mult)
                    nc.sync.dma_start_transpose(hidT[:, :, t * 128:(t + 1) * 128],
                                                tmp[:])
                # gamma/beta fused into gelu via per-partition scale/bias;
                # per n-half so w2 can start after the first 4 transposes
                for nh in range(2):
                    for ki in range(KM):
                        sl = hidT[:, ki, nh * 512:(nh + 1) * 512]
                        nc.scalar.activation(sl, sl, Act.Gelu,
                                             bias=bln_sb[:, ki:ki + 1],
                                             scale=gln_sb[:, ki:ki + 1])
                St[c]["hidT"] = hidT

            # ---------- stage 3: w2 GEMM -> mixT16 (chunked by m for overlap) ----------
            def st3_mm(c, m_lo, m_hi):
                hidT = St[c]["hidT"]
                if "mixT16" not in St[c]:
                    St[c]["mixT16"] = ws.tile([128, KH, CHUNK], f16, tag="B", bufs=2, name="mixT16")
                mixT16 = St[c]["mixT16"]
                if "mix8" not in St[c]:
                    St[c]["mix8"] = ws.tile([128, KH, CHUNK], f8, tag="E8",
                                            bufs=2, name="mix8")
                mix8 = St[c]["mix8"]
                for m in range(m_lo, m_hi):
                    wide = ps2.tile([128, CHUNK], f32, tag="wide",
                                    name="w2wide") if c == 0 else None
                    for n in range(2):
                        mps = wide[:, n * 512:(n + 1) * 512] if c == 0 else \
                            ps.tile([128, 512], f32, tag="mm")
                        for ki in range(KM):
                            nc.tensor.matmul(mps[:], w2_sb[:, ki, m * 128:(m + 1) * 128],
                                             hidT[:, ki, n * 512:(n + 1) * 512],
                                             start=(ki == 0), stop=(ki == KM - 1))
                        if c == 0:
                            nc.scalar.activation(
                                mixT16[:, m, n * 512:(n + 1) * 512], mps[:],
                                Act.Identity, bias=b2r_sb[:, m:m + 1],
                                scale=SC_RES)
                        else:
                            nc.vector.tensor_scalar(
                                mixT16[:, m, n * 512:(n + 1) * 512],
                                mps[:], b2_sb[:, m:m + 1], SC_RES,
                                op0=Alu.add, op1=Alu.mult)
                        if c == 0:
                            nc.vector.tensor_scalar(
                                mix8[:, m, n * 512:(n + 1) * 512],
                                mps[:], b2_sb[:, m:m + 1], SC_MIX8,
                                op0=Alu.add, op1=Alu.mult)
                        else:
                            nc.gpsimd.tensor_scalar(
                                mix8[:, m, n * 512:(n + 1) * 512],
                                mixT16[:, m, n * 512:(n + 1) * 512],
                                SC_MIX8 / SC_RES, None, op0=Alu.mult)

            def st3_post(c):
                mixT16 = St[c]["mixT16"]
                mixN = ws.tile([128, NT, H], f16, tag="mixN", bufs=2)
                for m in range(KH):
                    nc.sync.dma_start_transpose(mixN[:, :, m * 128:(m + 1) * 128],
                                                mixT16[:, m, :])
                St[c]["mixN"] = mixN

            # ---------- stage 4: q/k/v projections (fp8 double-row) ----------
            def st4(c):
                mix8 = St[c]["mix8"]
                qT8 = ws.tile([128, KH, CHUNK], f8, tag="Q8", bufs=1)
                kT8 = ws.tile([128, KH, CHUNK], f8, tag="K8", bufs=1)
                for (dst, wsb, on_act) in ((qT8, wq_sb, True), (kT8, wk_sb, False)):
                    for m in range(KH):
                        qps = ps2.tile([128, CHUNK], f32, tag="wide")
                        for n in range(2):
                            for kj in range(KP):
                                nc.tensor.matmul(
                                    qps[:, n * 512:(n + 1) * 512],
                                    wsb[:, 2 * kj:2 * kj + 2, m * 128:(m + 1) * 128],
                                    mix8[:, 2 * kj:2 * kj + 2, n * 512:(n + 1) * 512],
                                    start=(kj == 0), stop=(kj == KP - 1),
                                    perf_mode=DR)
                        for n in range(2):
                            half = qps[:, n * 512:(n + 1) * 512]
                            osl = dst[:, m, n * 512:(n + 1) * 512]
                            if (n == 0) == on_act:
                                nc.scalar.mul(osl, half, 1.0 / SC_W)
                            else:
                                nc.vector.tensor_scalar(osl, half, 1.0 / SC_W,
                                                        None, op0=Alu.mult)
                vN8 = ws.tile([128, NT, H], f8, tag="V8", bufs=1)
                for t in range(NT):
                    vps = ps2.tile([128, CHUNK], f32, tag="wide")
                    for n in range(2):
                        for kj in range(KP):
                            nc.tensor.matmul(
                                vps[:, n * 512:(n + 1) * 512],
                                mix8[:, 2 * kj:2 * kj + 2, t * 128:(t + 1) * 128],
                                wv_sb[:, 2 * kj:2 * kj + 2, n * 512:(n + 1) * 512],
                                start=(kj == 0), stop=(kj == KP - 1),
                                perf_mode=DR)
                    nc.scalar.mul(vN8[:, t, :512], vps[:, :512], 1.0 / SC_W)
                    nc.vector.tensor_scalar(vN8[:, t, 512:], vps[:, 512:],
                                            1.0 / SC_W, None, op0=Alu.mult)
                St[c]["qT8"], St[c]["kT8"], St[c]["vN8"] = qT8, kT8, vN8

            # ---------- stage 5: attention ----------
            def sc_exp(c, h):
                qT8, kT8 = St[c]["qT8"], St[c]["kT8"]
                et8 = ws.tile([128, KH, CHUNK], f8, tag="E8", bufs=2)
                for kt in range(NT):
                    stp = ps2.tile([128, CHUNK], f32, tag="wide")
                    for qn in range(2):
                        nc.tensor.matmul(
                            stp[:, qn * 512:(qn + 1) * 512],
                            kT8[:, 2 * h:2 * h + 2, kt * 128:(kt + 1) * 128],
                            qT8[:, 2 * h:2 * h + 2, qn * 512:(qn + 1) * 512],
                            start=True, stop=True, perf_mode=DR)
                    # psum holds (4q.4k)=16*qk
                    nc.scalar.activation(et8[:, kt, :], stp[:], Act.Exp,
                                         scale=float(HD ** -0.5 / 16.0))
                St[c]["et8"] = et8

            def pv(c, h, tail=None):
                et8, vN8 = St[c]["et8"], St[c]["vN8"]
                if "ocat" not in St[c]:
                    St[c]["ocat"] = ws.tile([128, NT, H], f16, tag="A", bufs=2, name="ocat")
                ocat = St[c]["ocat"]
                last = h == NUM_HEADS - 1
                if last:
                    St[c]["otc"] = ws.tile([128, KH, CHUNK], f16, tag="A",
                                           bufs=2, name="otc")
                    St[c]["otc8"] = ws.tile([128, KH, CHUNK], f8, tag="O8y",
                                            bufs=1, name="otc8")
                for qt in range(NT):
                    ovp = ps.tile([128, 512], f32, tag="mm")
                    for kj in range(KP):
                        nc.tensor.matmul(ovp[:, :HD],
                                         et8[:, 2 * kj:2 * kj + 2,
                                             qt * 128:(qt + 1) * 128],
                                         vN8[:, 2 * kj:2 * kj + 2,
                                             h * HD:(h + 1) * HD],
                                         start=(kj == 0), stop=(kj == KP - 1),
                                         perf_mode=DR)
                        # denominator column rides in the same bank; kj==0
                        # start=True above cleared it, so keep start=False.
                        nc.tensor.matmul(ovp[:, HD:HD + 1],
                                         et8[:, 2 * kj:2 * kj + 2,
                                             qt * 128:(qt + 1) * 128],
                                         ones2[:],
                                         start=False, stop=(kj == KP - 1),
                                         perf_mode=DR, skip_group_check=True)
                    rq = sm.tile([128, 1], f32, tag="rq")
                    nc.vector.reciprocal(rq[:], ovp[:, HD:HD + 1])
                    # ocat = SC_OT*SC_MIX8*o (ones=1/32 baked the 32x)
                    if last and c == 1:
                        nc.scalar.activation(ocat[:, qt, h * HD:(h + 1) * HD],
                                             ovp[:, :HD], Act.Copy, scale=rq[:])
                    else:
                        nc.vector.tensor_scalar(ocat[:, qt, h * HD:(h + 1) * HD],
                                                ovp[:, :HD], rq[:], None,
                                                op0=Alu.mult)
                    if last:
                        otc, otc8 = St[c]["otc"], St[c]["otc8"]
                        blk = slice(qt * 128, (qt + 1) * 128)
                        nc.sync.dma_start_transpose(otc[:, :, blk], ocat[:, qt, :])
                        # fp8 casts per column block on idle engines
                        for piece in range(2):
                            rows = slice(4 * piece, 4 * piece + 4)
                            if qt % 2 == 0:
                                nc.scalar.copy(otc8[:, rows, blk], otc[:, rows, blk])
                            else:
                                nc.gpsimd.tensor_copy(otc8[:, rows, blk],
                                                      otc[:, rows, blk])
                        if tail is not None:
                            tail(qt)

            # ---------- stage 6 pre: transpose ocat + fp8 cast ----------
            def st6_pre(c):
                St[c]["zT"] = ws.tile([128, KH, CHUNK], f16, tag="A", bufs=2,
                                      name="zT")

            # ---------- stage 6: wo + residual(PE) + LN2 + z, per token tile ----------
            def wo_ln2(c, t_lo, t_hi, do_zt=True):
                otc8, mixN = St[c]["otc8"], St[c]["mixN"]
                zT = St[c].get("zT")
                if "z" not in St[c]:
                    St[c]["z"] = ws.tile([128, NT, H], f16, tag="B", bufs=2, name="z")
                z = St[c]["z"]
                for t in range(t_lo, t_hi):
                    ops2 = ps2.tile([128, CHUNK], f32, tag="wide")
                    for n in range(2):
                        for fi in range(KP):
                            nc.tensor.matmul(
                                ops2[:, n * 512:(n + 1) * 512],
                                otc8[:, 2 * fi:2 * fi + 2, t * 128:(t + 1) * 128],
                                wo_sb[:, 2 * fi:2 * fi + 2, n * 512:(n + 1) * 512],
                                start=(fi == 0), stop=False, perf_mode=DR)
                        # residual: += I.T @ mixN on the PE (both SC_RES-scaled)
                        nc.tensor.matmul(ops2[:, n * 512:(n + 1) * 512], id_sb[:],
                                         mixN[:, t, n * 512:(n + 1) * 512],
                                         start=False, stop=True)
                    st6b = sm.tile([128, 2, 6], f32, tag="st6b")
                    for half in range(2):
                        nc.vector.bn_stats(st6b[:, half, :],
                                           ops2[:, half * 512:(half + 1) * 512])
                    mv2 = sm.tile([128, 2], f32, tag="mv")
                    nc.vector.bn_aggr(mv2[:], st6b[:])
                    sq2 = sm.tile([128, 1], f32, tag="sq")
                    nc.scalar.activation(sq2[:], mv2[:, 1:2], Act.Sqrt,
                                         bias=eps2_sb[:])
                    iv2 = sm.tile([128, 1], f32, tag="iv")
                    nc.vector.reciprocal(iv2[:], sq2[:])
                    if c == 0:
                        nc.vector.tensor_scalar(z[:, t, :], ops2[:], mv2[:, 0:1],
                                                iv2[:], op0=Alu.subtract,
                                                op1=Alu.mult)
                    else:
                        nmi = sm.tile([128, 1], f32, tag="nmi")
                        nc.vector.tensor_scalar(nmi[:], mv2[:, 0:1], iv2[:], -1.0,
                                                op0=Alu.mult, op1=Alu.mult)
                        nc.scalar.activation(z[:, t, :], ops2[:], Act.Identity,
                                             bias=nmi[:], scale=iv2[:])
                    if do_zt:
                        nc.sync.dma_start_transpose(
                            zT[:, :, t * 128:(t + 1) * 128], z[:, t, :])

            def wo_ln2a(c, t_lo, t_hi):
                # wo GEMM + residual + stats + (res - mean) into z; the
                # iv scale is applied later by wo_ln2b so the Act sqrts can
                # run as one contiguous batch (one table load, not eight).
                otc8, mixN = St[c]["otc8"], St[c]["mixN"]
                if "z" not in St[c]:
                    St[c]["z"] = ws.tile([128, NT, H], f16, tag="B", bufs=2, name="z")
                if "mv2s" not in St[c]:
                    St[c]["mv2s"] = ws.tile([128, NT, 2], f32, tag="MVS",
                                            bufs=1, name="mv2s")
                z, mv2s = St[c]["z"], St[c]["mv2s"]
                for t in range(t_lo, t_hi):
                    ops2 = ps2.tile([128, CHUNK], f32, tag="wide")
                    for n in range(2):
                        for fi in range(KP):
                            nc.tensor.matmul(
                                ops2[:, n * 512:(n + 1) * 512],
                                otc8[:, 2 * fi:2 * fi + 2, t * 128:(t + 1) * 128],
                                wo_sb[:, 2 * fi:2 * fi + 2, n * 512:(n + 1) * 512],
                                start=(fi == 0), stop=False, perf_mode=DR)
                        nc.tensor.matmul(ops2[:, n * 512:(n + 1) * 512], id_sb[:],
                                         mixN[:, t, n * 512:(n + 1) * 512],
                                         start=False, stop=True)
                    st6b = sm.tile([128, 2, 6], f32, tag="st6b")
                    for half in range(2):
                        nc.vector.bn_stats(st6b[:, half, :],
                                           ops2[:, half * 512:(half + 1) * 512])
                    nc.vector.bn_aggr(mv2s[:, t, :], st6b[:])
                    nc.vector.tensor_scalar(z[:, t, :], ops2[:], mv2s[:, t, 0:1],
                                            None, op0=Alu.subtract)

            def wo_ln2b(c):
                z, mv2s, zT = St[c]["z"], St[c]["mv2s"], St[c]["zT"]
                iv2s = ws.tile([128, NT, 1], f32, tag="IVS", bufs=2, name="iv2s")
                for t in range(NT):
                    sq = sm.tile([128, 1], f32, tag="sq")
                    nc.scalar.activation(sq[:], mv2s[:, t, 1:2], Act.Sqrt,
                                         bias=eps2_sb[:])
                    nc.vector.reciprocal(iv2s[:, t, :], sq[:])
                for t in range(NT):
                    nc.vector.tensor_scalar(z[:, t, :], z[:, t, :],
                                            iv2s[:, t, :], None, op0=Alu.mult)
                    nc.sync.dma_start_transpose(zT[:, :, t * 128:(t + 1) * 128],
                                                z[:, t, :])

            def zt7(c, t_lo, t_hi):
                zT, z = St[c]["zT"], St[c]["z"]
                for t in range(t_lo, t_hi):
                    nc.sync.dma_start_transpose(zT[:, :, t * 128:(t + 1) * 128],
                                                z[:, t, :])
                st7(c, t_lo, t_hi)

            # ---------- stage 7: output projection (fp16) ----------
            def st7(c, t_lo, t_hi):
                zT = St[c]["zT"]
                if "ych" not in St[c]:
                    St[c]["ych"] = ws.tile([128, NT, G], f32, tag="O8y", bufs=1, name="ych")
                ych = St[c]["ych"]
                for t in range(t_lo, t_hi, 2):
                    yps = ps2.tile([128, CHUNK], f32, tag="wide")
                    for tt in (t, t + 1):
                        if tt >= t_hi:
                            continue
                        off = (tt - t) * 512
                        for fi in range(KH):
                            nc.tensor.matmul(yps[:, off:off + G],
                                             zT[:, fi, tt * 128:(tt + 1) * 128],
                                             Gw["t"][:, fi, 0, :],
                                             start=(fi == 0), stop=(fi == KH - 1))
                        nc.vector.tensor_add(ych[:, tt, :], yps[:, off:off + G],
                                             bw_sb[:])

            def yout(c):
                ych = St[c]["ych"]
                for hh in range(2):
                    nc.sync.dma_start(
                        y.ap()[c, hh * 512:(hh + 1) * 512, :].rearrange(
                            "(t p) g -> p t g", p=128),
                        ych[:, hh * 4:(hh + 1) * 4, :])

            import itertools
            MARKS = []

            def mark(label):
                # all_instructions grows in emission order; len = next index
                MARKS.append((label, len(list(nc.all_instructions()))))
            _build.MARKS = MARKS

            # ================= emission schedule (software pipeline) =================
            mark("st1(0)"); st1(0)
            mark("st1(1)"); st1(1)
            mark("load_weights()"); load_weights()
            mark("st2a(0)"); st2a(0)
            mark("st2b(0)"); st2b(0)
            mark("st2a(1)"); st2a(1)
            mark("st3_mm(0,0,KH)"); st3_mm(0, 0, KH)
            mark("st3_post(0)"); st3_post(0)
            mark("st4(0)"); st4(0)
            mark("st2b(1)"); st2b(1)
            # chunk0 attention; chunk1 mixer GEMMs fill the exp-bound PE gaps
            mark("sc_exp(0,0)"); sc_exp(0, 0); mark("st3_mm(1,0,4)"); st3_mm(1, 0, 4); mark("pv(0,0)"); pv(0, 0)
            mark("sc_exp(0,1)"); sc_exp(0, 1); mark("st3_mm(1,4,KH)"); st3_mm(1, 4, KH); mark("pv(0,1)"); pv(0, 1)
            mark("sc_exp(0,2)"); sc_exp(0, 2); mark("st3_post(1)"); st3_post(1); load_gw(); mark("pv(0,2)"); pv(0, 2)
            mark("sc_exp(0,3)"); sc_exp(0, 3); mark("pv(0,3)"); pv(0, 3)
            mark("st4(1)"); st4(1)
            mark("st6_pre(0)"); st6_pre(0)
            # chunk0 output stages; chunk1 attention fills the gaps
            mark("sc_exp(1,0)"); sc_exp(1, 0); mark("wo_ln2(0,0,2)"); wo_ln2(0, 0, 2); mark("pv(1,0)"); pv(1, 0)
            mark("sc_exp(1,1)"); sc_exp(1, 1); mark("wo_ln2(0,2,4)"); wo_ln2(0, 2, 4); mark("pv(1,1)"); pv(1, 1)
            mark("sc_exp(1,2)"); sc_exp(1, 2); mark("wo_ln2(0,4,6)"); wo_ln2(0, 4, 6); mark("st7(0,0,2)"); st7(0, 0, 2); mark("pv(1,2)"); pv(1, 2)
            mark("sc_exp(1,3)"); sc_exp(1, 3)
            mark("wo_ln2(0,6,NT)"); wo_ln2(0, 6, NT)
            mark("st7(0,2,NT)"); st7(0, 2, NT); mark("yout(0)"); yout(0)

            def tail1(qt):
                # wo+LN2 of chunk1 lagging two blocks behind the cast pipeline
                if qt >= 2:
                    mark(f"wo_ln2(1,{qt-2})"); wo_ln2(1, qt - 2, qt - 1, do_zt=False)
            mark("pv(1,3)"); pv(1, 3, tail=tail1)
            mark("st6_pre(1)"); st6_pre(1)
            mark("wo_ln2(1,6,7)"); wo_ln2(1, 6, 7, do_zt=False)
            mark("wo_ln2(1,7,NT)"); wo_ln2(1, 7, NT, do_zt=False)
            mark("zt7(1,0,2)"); zt7(1, 0, 2)
            mark("zt7(1,2,4)"); zt7(1, 2, 4)
            mark("zt7(1,4,6)"); zt7(1, 4, 6)
            mark("zt7(1,6,NT)"); zt7(1, 6, NT)
            mark("yout(1)"); yout(1)

    nc.compile()
    return nc


def _get_compiled():
    global _COMPILED
    if _COMPILED is None:
        _COMPILED = _build()
    return _COMPILED


def _prep_inputs(inputs):
    f32 = np.float32

    def a(name):
        return np.asarray(inputs[name], dtype=f32)

    x = a("x")
    mw = a("mother_wavelets")
    scales = a("scales")
    norm = np.sqrt(np.sum(mw ** 2, axis=2, keepdims=True))
    kern = (mw / np.maximum(norm, 1e-12)) * (1.0 / (1.0 + np.exp(-scales)))
    kern = kern[0, :, :, 0]                      # (W, H)
    kernT = np.ascontiguousarray(kern.T).astype(FP16)

    w1a = np.concatenate([a("mix_w1"), a("mix_b1")[None, :]], axis=0).astype(FP16)
    gln = np.ascontiguousarray(a("mix_ln_g").reshape(KM, 128).T).astype(f32)
    bln = np.ascontiguousarray(a("mix_ln_b").reshape(KM, 128).T).astype(f32)
    w2 = a("mix_w2").astype(FP16)
    b2c = np.ascontiguousarray(a("mix_b2").reshape(KH, 128).T).astype(f32)
    b2r = (b2c * SC_RES).astype(f32)
    gw = (a("out_ln_g")[:, None] * a("out_w")).astype(FP16)
    bw_vec = a("out_ln_b") @ a("out_w") + a("out_b")
    bw = np.tile(bw_vec[None, :], (128, 1)).astype(FP16)

    shared = {
        "kernt": kernT, "w1a": w1a, "gln": gln, "bln": bln, "w2": w2,
        "b2c": b2c, "b2r": b2r,
        "wq8": (a("wq") * SC_W).astype(FP8), "wk8": (a("wk") * SC_W).astype(FP8),
        "wv8": (a("wv") * SC_W).astype(FP8), "wo8": (a("wo") * SC_W).astype(FP8),
        "ident": np.eye(128, dtype=FP16),
        "gw": gw, "bw": bw,
    }

    xc = x.reshape(N_CHUNKS, CHUNK, H)
    xt_all = np.ascontiguousarray(xc.transpose(0, 2, 1)).astype(FP16)  # (16, H, CHUNK)
    in_maps = []
    for core in range(N_CORES):
        m = dict(shared)
        m["xt"] = np.ascontiguousarray(xt_all[core * CPC:(core + 1) * CPC])
        in_maps.append(m)
    return in_maps


def kernel(**inputs) -> np.ndarray:
    from concourse.bass_utils import run_bass_kernel_spmd

    nc = _get_compiled()
    in_maps = _prep_inputs(inputs)
    res = run_bass_kernel_spmd(nc, in_maps, core_ids=list(range(N_CORES)))
    out = np.concatenate([r["y"] for r in res.results], axis=0)  # (16, CHUNK, G)
    return out.reshape(B, S, G).astype(np.float32)

